# revision 1
# baseline (speedup 1.0000x reference)
"""CARAFE kernel for Trainium2 (8 NeuronCores, batch-parallel).

Reference computation per image:
  R = relu(conv1x1(x, w_compress, b_compress))          [48, 128, 128]
  E = conv3x3(R, w_encoder, b_encoder, pad=1)           [100, 128, 128]
  Y = softmax over k of E.reshape(4, 25, H, W)          (s, k, h, w)
  out[s,c,h,w] = sum_k Y[s,k,h,w] * xpad[c, h+dy, w+dx] (k=(dy,dx), 5x5, pad 2)
  pixel-shuffle: out_ref[s*16 + c//4, 2h + (c//2)%2, 2w + c%2] = out[s,c,h,w]

Mapping:
  - conv1x1 / conv3x3 / softmax-denominator: TensorE matmuls (channel-major),
    exp on ScalarE.  Biases folded in via a constant ones row (K=65 / K=49).
  - softmax normalization folded into the F-transpose epilogue on ScalarE.
  - The per-pixel weighted patch sum runs on VectorE in pixel-major layout
    [128 h-partitions, (c, w) free]: one mult + one add per (s, k) tap with
    the per-pixel weight broadcast along c via a free-dim step-0 AP.
    dy taps select one of five partition-shifted copies of X^T (built by
    DMA; compute engines cannot shift partitions), dx taps are free-dim
    offsets into a w-padded buffer (zero padding gives the conv edge
    semantics for free).
"""

import sys

import numpy as np

sys.path.insert(0, "/opt/trn_rl_repo")

import concourse.bass as bass
import concourse.mybir as mybir
import concourse.tile as tile
from concourse import bacc
from concourse.masks import make_identity

F32 = mybir.dt.float32

H = 128
W = 128
C = 64
M = 48  # compressed channels
S2 = 4  # scale_factor**2
K2 = 25  # k_up**2
SK = 100
HW = H * W
WPAD = W + 4  # w-padded pixel-major buffers
N_CORES = 8


def _ap(t, extra_off, dims):
    """Raw AP on a tile handle `t` with free-offset `extra_off` (elements)
    and explicit [step, count] dims (dims[0] is the partition dim)."""
    base = t[:]
    return bass.AP(tensor=base.tensor, offset=base.offset + extra_off, ap=dims)


class _Pool:
    """Manually scoped tile pool."""

    def __init__(self, tc, **kw):
        self._cm = tc.tile_pool(**kw)
        self.pool = self._cm.__enter__()
        self._n = 0

    def tile(self, *a, tag=None, **kw):
        self._n += 1
        t = tag or f"t{self._n}"
        return self.pool.tile(*a, tag=t, name=t, **kw)

    def close(self):
        self._cm.__exit__(None, None, None)


def build_program(debug=False, reps=1):
    nc = bacc.Bacc("TRN2", target_bir_lowering=False, debug=False)

    xin = nc.dram_tensor("xin", [C, HW], F32, kind="ExternalInput")
    w1t = nc.dram_tensor("w1t", [C + 1, M], F32, kind="ExternalInput")
    wet = nc.dram_tensor("wet", [M + 1, 9 * SK], F32, kind="ExternalInput")
    sones = nc.dram_tensor("sones", [SK, S2], F32, kind="ExternalInput")
    onesr = nc.dram_tensor("onesr", [1, 130 * 130], F32, kind="ExternalInput")
    zer = nc.dram_tensor("zer", [2, C * WPAD], F32, kind="ExternalInput")
    out = nc.dram_tensor("out", [C, 2 * H, 2 * W], F32, kind="ExternalOutput")
    dbg = {}
    if debug:
        dbg["R"] = nc.dram_tensor("dbgR", [M, HW], F32, kind="ExternalOutput")
        dbg["F"] = nc.dram_tensor("dbgF", [SK, HW], F32, kind="ExternalOutput")
        dbg["FR"] = nc.dram_tensor("dbgFR", [128, SK * W], F32, kind="ExternalOutput")
        dbg["XT"] = nc.dram_tensor("dbgXT", [128, C * WPAD], F32, kind="ExternalOutput")

    with tile.TileContext(nc) as tc:
        cp = _Pool(tc, name="consts", bufs=1)
        pp = _Pool(tc, name="persist", bufs=1)

        w1t_sb = cp.tile([C + 1, M], F32)
        nc.sync.dma_start(w1t_sb[:], w1t.ap())
        wet_sb = cp.tile([M + 1, 9 * SK], F32)
        nc.sync.dma_start(wet_sb[:], wet.ap())
        sones_sb = cp.tile([SK, S2], F32)
        nc.sync.dma_start(sones_sb[:], sones.ap())
        ident = cp.tile([128, 128], F32)
        make_identity(nc, ident[:])
        rzbuf = pp.tile([128, S2 * W], F32)
        xt_base = pp.tile([128, C * WPAD], F32)

        for _rep in range(reps):
            # ---- load x (+ ones row) ----
            px = _Pool(tc, name="px", bufs=1)
            x_aug = px.tile([C + 1, HW], F32)
            nc.sync.dma_start(x_aug[0:C, :], xin.ap())
            nc.sync.dma_start(
                _ap(x_aug, C * HW, [[HW, 1], [1, HW]]), onesr.ap()[:, 0:HW]
            )

            # ---- X^T via PE transpose -> XT_base [128(h), (c, WPAD)] ----
            nc.vector.memset(xt_base[:], 0.0)
            psX = _Pool(tc, name="psX", bufs=2, space="PSUM")
            for w in range(W):
                psx = psX.tile([128, C], F32, tag="psx")
                nc.tensor.transpose(
                    psx[:], _ap(x_aug, w, [[HW, C], [W, H]]), ident[0:C, 0:C]
                )
                nc.scalar.copy(
                    _ap(xt_base, 2 + w, [[C * WPAD, 128], [WPAD, C]]), psx[:]
                )
            psX.close()
            if debug:
                nc.sync.dma_start(dbg["XT"].ap(), xt_base[:])

            # ---- pass 1 (banded): conv1x1 -> relu -> r_band; conv3x3 -> exp -> f_dram; Z ----
            f_dram = nc.dram_tensor(f"fstage{_rep}", [SK, HW], F32, kind="Internal")
            BH = 32  # band height
            RB = BH + 2  # rows held per band (1-halo each side)
            RBF = RB * 130
            pband = _Pool(tc, name="pband", bufs=2)
            psA = _Pool(tc, name="psA", bufs=2, space="PSUM")
            psB = _Pool(tc, name="psB", bufs=2, space="PSUM")
            psBsb = _Pool(tc, name="psBsb", bufs=2)

            def conv1x1_rows(r_band, h0, nrows, loc0):
                """conv1x1+relu for image rows [h0, h0+nrows) into band-local row loc0."""
                ps1 = psA.tile([M, 512], F32, tag="ps1")
                nc.tensor.matmul(
                    ps1[:, 0 : nrows * W],
                    w1t_sb[:],
                    x_aug[:, h0 * W : (h0 + nrows) * W],
                    start=True,
                    stop=True,
                )
                nc.scalar.activation(
                    _ap(r_band, loc0 * 130 + 1, [[RBF, M], [130, nrows], [1, W]]),
                    ps1[:, 0 : nrows * W],
                    mybir.ActivationFunctionType.Relu,
                )

            for b in range(4):
                r_band = pband.tile([M + 1, RBF], F32, tag="rband")
                nc.gpsimd.memset(r_band[:], 0.0)
                nc.sync.dma_start(
                    _ap(r_band, M * RBF, [[RBF, 1], [1, RBF]]), onesr.ap()[:, 0:RBF]
                )
                # band covers image rows 32b-1 .. 32b+32 at band-local rows 0..33
                if b > 0:
                    conv1x1_rows(r_band, 32 * b - 1, 1, 0)
                for j in range(8):
                    conv1x1_rows(r_band, 32 * b + 4 * j, 4, 1 + 4 * j)
                if b < 3:
                    conv1x1_rows(r_band, 32 * b + 32, 1, 33)
                for j in range(8):
                    ps2 = psB.tile([SK, 512], F32, tag="ps2")
                    for t in range(9):
                        ty, tx = divmod(t, 3)
                        nc.tensor.matmul(
                            ps2[:],
                            wet_sb[:, t * SK : (t + 1) * SK],
                            _ap(
                                r_band,
                                (4 * j + ty) * 130 + tx,
                                [[RBF, M + 1], [130, 4], [1, W]],
                            ),
                            start=(t == 0),
                            stop=(t == 8),
                        )
                    fc = psBsb.tile([SK, 512], F32, tag="fc")
                    nc.scalar.activation(
                        fc[:], ps2[:], mybir.ActivationFunctionType.Exp
                    )
                    n = 8 * b + j
                    nc.sync.dma_start(
                        f_dram.ap()[:, n * 512 : (n + 1) * 512], fc[:]
                    )
                    psz = psB.tile([S2, 512], F32, tag="psz")
                    nc.tensor.matmul(
                        psz[:], sones_sb[:], fc[:], start=True, stop=True
                    )
                    zc = psBsb.tile([S2, 512], F32, tag="zc")
                    nc.scalar.copy(zc[:], psz[:])
                    # scatter Z into rzbuf [128(h), (s, w)]: rows 4n..4n+3
                    for s in range(S2):
                        nc.sync.dma_start(
                            _ap(
                                rzbuf,
                                4 * n * (S2 * W) + s * W,
                                [[S2 * W, 4], [1, W]],
                            ),
                            _ap(zc, s * 512, [[512, 1], [W, 4], [1, W]]),
                        )
            psBsb.close()
            psB.close()
            psA.close()
            pband.close()
            px.close()

            nc.vector.reciprocal(rzbuf[:], rzbuf[:])

            # ---- pass 2: reload F, transposes ----
            pfr = _Pool(tc, name="pfr", bufs=1)
            fr = pfr.tile([128, SK * W], F32)
            pf = _Pool(tc, name="pf", bufs=1)
            f_sb = pf.tile([SK, HW], F32)
            nc.sync.dma_start(f_sb[:], f_dram.ap())
            if debug:
                nc.sync.dma_start(dbg["F"].ap(), f_sb[:])


            # ---- F^T transposes + softmax-normalize -> FR [128(h), (sk, w)] ----
            psF = _Pool(tc, name="psF", bufs=2, space="PSUM")
            for w in range(W):
                pst = psF.tile([128, SK], F32, tag="pst")
                nc.tensor.transpose(
                    pst[:], _ap(f_sb, w, [[HW, SK], [W, H]]), ident[0:SK, 0:SK]
                )
                for s in range(S2):
                    nc.scalar.activation(
                        _ap(fr, (s * K2) * W + w, [[SK * W, 128], [W, K2]]),
                        pst[:, s * K2 : (s + 1) * K2],
                        mybir.ActivationFunctionType.Copy,
                        scale=rzbuf[:, s * W + w : s * W + w + 1],
                    )
            psF.close()
            pf.close()
            if debug:
                nc.sync.dma_start(dbg["FR"].ap(), fr[:])

            # ---- per-pixel patch sum on VectorE ----
            WHF = W // 2  # 64 output w per half
            XF = C * (WHF + 4)
            xtp = _Pool(tc, name="xtd", bufs=2)
            accp = _Pool(tc, name="acc", bufs=1)
            tmpp = _Pool(tc, name="tmp", bufs=1)
            acc2p = _Pool(tc, name="acc2", bufs=1)
            for half in range(2):
                for s in range(S2):
                    acc = accp.tile([128, C * WHF], F32, tag="acc")
                    for dy in range(-2, 3):
                        xtd = xtp.tile([128, XF], F32, tag="xtd")
                        p0, p1 = max(0, -dy), 128 - max(0, dy)
                        # body: partition-shifted, w-windowed copy of XT_base
                        nc.sync.dma_start(
                            _ap(xtd, p0 * XF, [[XF, p1 - p0], [1, XF]]),
                            _ap(
                                xt_base,
                                (p0 + dy) * (C * WPAD) + half * WHF,
                                [[C * WPAD, p1 - p0], [WPAD, C], [1, WHF + 4]],
                            ),
                        )
                        if p0 > 0:  # top halo rows <- zeros
                            nc.sync.dma_start(
                                _ap(xtd, 0, [[XF, p0], [1, XF]]), zer.ap()[0:p0, 0:XF]
                            )
                        if p1 < 128:  # bottom halo rows <- zeros
                            nc.sync.dma_start(
                                _ap(xtd, p1 * XF, [[XF, 128 - p1], [1, XF]]),
                                zer.ap()[0 : 128 - p1, 0:XF],
                            )
                        for dx in range(-2, 3):
                            k = (dy + 2) * 5 + (dx + 2)
                            sk = s * K2 + k
                            in0 = _ap(
                                xtd, 2 + dx, [[XF, 128], [WHF + 4, C], [1, WHF]]
                            )
                            in1 = _ap(
                                fr,
                                sk * W + half * WHF,
                                [[SK * W, 128], [0, C], [1, WHF]],
                            )
                            dst3 = _ap(acc, 0, [[C * WHF, 128], [WHF, C], [1, WHF]])
                            if k == 0:
                                nc.vector.tensor_mul(dst3, in0, in1)
                            else:
                                tmp = tmpp.tile([128, C * WHF], F32, tag="tmp")
                                t3 = _ap(tmp, 0, [[C * WHF, 128], [WHF, C], [1, WHF]])
                                nc.vector.tensor_mul(t3, in0, in1)
                                nc.vector.tensor_add(acc[:], acc[:], tmp[:])
                    # reshuffle (c, w) -> (c4, c2, w, c1) and DMA out
                    acc2 = acc2p.tile([128, C * WHF], F32, tag="acc2")
                    nc.scalar.copy(
                        acc2[:].rearrange(
                            "p (a b w d) -> p a b w d", a=16, b=2, w=WHF
                        ),
                        _ap(
                            acc,
                            0,
                            [
                                [C * WHF, 128],
                                [4 * WHF, 16],
                                [2 * WHF, 2],
                                [1, WHF],
                                [WHF, 2],
                            ],
                        ),
                    )
                    # out[s*16+c4, 2h+c2, 2*(half*64+w)+c1]; split per c2
                    for c2 in range(2):
                        dst = bass.AP(
                            tensor=out,
                            offset=(s * 16) * (4 * HW) + c2 * (2 * W) + half * W,
                            ap=[
                                [2 * (2 * W), 128],  # h -> row 2h
                                [4 * HW, 16],  # c4
                                [1, 2 * WHF],  # (w, c1) contiguous
                            ],
                        )
                        src = _ap(
                            acc2,
                            c2 * (2 * WHF),
                            [[C * WHF, 128], [4 * WHF, 16], [1, 2 * WHF]],
                        )
                        nc.sync.dma_start(dst, src)
            acc2p.close()
            tmpp.close()
            accp.close()
            xtp.close()
            pfr.close()
        pp.close()
        cp.close()
    nc.compile()
    return nc, dbg


def host_inputs(x_img, w_compress, b_compress, w_encoder, b_encoder):
    """Per-core input map for one image [C, H, W]."""
    w1t = np.concatenate(
        [w_compress[:, :, 0, 0].T, b_compress[None, :]], axis=0
    ).astype(np.float32)
    wet = np.zeros((M + 1, 9, SK), np.float32)
    for ty in range(3):
        for tx in range(3):
            wet[:M, ty * 3 + tx, :] = w_encoder[:, :, ty, tx].T
    wet[M, 4, :] = b_encoder
    son = np.zeros((SK, S2), np.float32)
    for s in range(S2):
        son[s * K2 : (s + 1) * K2, s] = 1.0
    return {
        "xin": np.ascontiguousarray(x_img.reshape(C, HW)).astype(np.float32),
        "w1t": w1t,
        "wet": wet.reshape(M + 1, 9 * SK),
        "sones": son,
        "onesr": np.ones((1, 130 * 130), np.float32),
        "zer": np.zeros((2, C * WPAD), np.float32),
    }


_CACHE = {}


def kernel(x, w_compress, b_compress, w_encoder, b_encoder):
    x = np.asarray(x, np.float32)
    if "nc" not in _CACHE:
        _CACHE["nc"], _ = build_program(debug=False)
    nc = _CACHE["nc"]
    in_maps = [
        host_inputs(
            x[i],
            np.asarray(w_compress, np.float32),
            np.asarray(b_compress, np.float32),
            np.asarray(w_encoder, np.float32),
            np.asarray(b_encoder, np.float32),
        )
        for i in range(N_CORES)
    ]
    from concourse.bass_utils import run_bass_kernel_spmd

    res = run_bass_kernel_spmd(nc, in_maps, core_ids=list(range(N_CORES)))
    return np.stack([res.results[i]["out"] for i in range(N_CORES)], axis=0)



# revision 9
# speedup vs baseline: 3.3361x; 3.3361x over previous
"""CARAFE kernel for Trainium2 (8 NeuronCores, batch-parallel), v2.

Reference computation per image (one per core):
  R = relu(conv1x1(x, w_compress, b_compress))          [48, 128, 128]
  E = conv3x3(R, w_encoder, b_encoder, pad=1)           [100, 128, 128]
  Y = softmax over k of E.reshape(4, 25, H, W)          (s, k, h, w)
  out[s,c,h,w] = sum_k Y[s,k,h,w] * xpad[c, h+dy, w+dx] (k=(dy,dx), 5x5, pad 2)
  pixel-shuffle: out_ref[s*16 + c//4, 2h + (c//2)%2, 2w + c%2] = out[s,c,h,w]

Mapping (all 16-bit datapaths; measured E range is [-3.2, 3.3] so fp16
holds exp(E) and all intermediates comfortably):
  - conv1x1 / conv3x3: fp16 TensorE matmuls over a zero-padded 130x130 R
    grid; biases via a constant ones row.  exp fused into the PSUM->SBUF
    copy on ScalarE.
  - F^T transpose and the softmax denominator in ONE matmul per w-column:
    the moving operand is [I_100 | S] where S sums each s-group of 25, so
    PSUM gets F^T columns and Z^T columns together.
  - softmax normalize: one reciprocal + one broadcast tensor_mul on fr.
  - patch sum on VectorE in fp16 (2x perf mode): pixel-major layout
    [128 h-partitions, (c, w)].  dx taps are free-dim offsets; odd dx uses
    a one-element-shifted copy so every operand stays 4B-aligned.  dy taps
    read partition-shifted copies of X^T (built by contiguous full-row
    SBUF->SBUF DMA); edge rows are handled by clamping the partition range
    of the mult/add (zero-padding contributes nothing).
  - pixel shuffle via a strided ScalarE copy (fp16->fp32) into
    (c4, r1, w, r2) order, then DMA with 2KB-contiguous runs.
"""

import sys

import numpy as np

sys.path.insert(0, "/opt/trn_rl_repo")

import concourse.bass as bass
import concourse.mybir as mybir
import concourse.tile as tile
from concourse import bacc

F32 = mybir.dt.float32
F16 = mybir.dt.float16

H = 128
W = 128
C = 64
M = 48  # compressed channels
S2 = 4  # scale_factor**2
K2 = 25  # k_up**2
SK = 100
HW = H * W
WP = 132  # padded row width in pixel-major x buffers
XF = C * WP  # 8448 free elems per partition
G = 130 * 130  # padded R grid
N_CORES = 8


def _ap(t, extra_off, dims):
    """Raw AP on a tile handle `t` with free-offset `extra_off` (elements)
    and explicit [step, count] dims (dims[0] is the partition dim)."""
    base = t[:]
    return bass.AP(tensor=base.tensor, offset=base.offset + extra_off, ap=dims)


class _Pool:
    """Manually scoped tile pool."""

    def __init__(self, tc, **kw):
        self._cm = tc.tile_pool(**kw)
        self.pool = self._cm.__enter__()
        self._n = 0

    def tile(self, *a, tag=None, **kw):
        self._n += 1
        t = tag or f"t{self._n}"
        return self.pool.tile(*a, tag=t, name=t, **kw)

    def close(self):
        self._cm.__exit__(None, None, None)


def build_program():
    nc = bacc.Bacc("TRN2", target_bir_lowering=False, debug=False)

    xin = nc.dram_tensor("xin", [C, HW], F32, kind="ExternalInput")
    w1te = nc.dram_tensor("w1te", [C + 1, M], F16, kind="ExternalInput")
    wete = nc.dram_tensor("wete", [M + 1, 9 * SK], F16, kind="ExternalInput")
    identc = nc.dram_tensor("identc", [C, C], F16, kind="ExternalInput")
    idents = nc.dram_tensor("idents", [SK, SK + S2], F16, kind="ExternalInput")
    ones16 = nc.dram_tensor("ones16", [1, G], F16, kind="ExternalInput")
    zer16 = nc.dram_tensor("zer16", [2, XF], F16, kind="ExternalInput")
    out = nc.dram_tensor("out", [C, 4 * HW], F32, kind="ExternalOutput")

    with tile.TileContext(nc) as tc:
        cp = _Pool(tc, name="consts", bufs=1)
        w1te_sb = cp.tile([C + 1, M], F16, tag="w1te")
        nc.sync.dma_start(w1te_sb[:], w1te.ap())
        wete_sb = cp.tile([M + 1, 9 * SK], F16, tag="wete")
        nc.sync.dma_start(wete_sb[:], wete.ap())
        identc_sb = cp.tile([C, C], F16, tag="identc")
        nc.sync.dma_start(identc_sb[:], identc.ap())
        idents_sb = cp.tile([SK, SK + S2], F16, tag="idents")
        nc.sync.dma_start(idents_sb[:], idents.ap())

        # persistent through the patch-sum phase
        pxe0 = _Pool(tc, name="pxe0", bufs=1)
        XE0 = pxe0.tile([128, XF], F16, tag="xe0")
        nc.gpsimd.memset(XE0[:], 0.0)
        pfr = _Pool(tc, name="pfr", bufs=1)
        fr = pfr.tile([128, SK * W], F16, tag="fr")
        prz = _Pool(tc, name="prz", bufs=1)
        rz = prz.tile([128, S2 * W], F32, tag="rz")

        # ---- load x (cast fp32->fp16 during DMA) + ones row ----
        pxa = _Pool(tc, name="pxa", bufs=1)
        x_aug = pxa.tile([C + 1, HW], F16, tag="xaug")
        nc.gpsimd.dma_start(x_aug[0:C, :], xin.ap())
        nc.sync.dma_start(
            _ap(x_aug, C * HW, [[HW, 1], [1, HW]]), ones16.ap()[:, 0:HW]
        )

        pf = _Pool(tc, name="pf", bufs=1)
        F = pf.tile([SK, HW], F16, tag="F")

        # ---- R grid: zero pad + ones row ----
        pr = _Pool(tc, name="pr", bufs=1)
        R = pr.tile([M + 1, G], F16, tag="R")
        nc.gpsimd.memset(R[:], 0.0)
        nc.sync.dma_start(_ap(R, M * G, [[G, 1], [1, G]]), ones16.ap())

        # ---- conv1x1 + relu into R interior ----
        psA = _Pool(tc, name="psA", bufs=2, space="PSUM")
        for j in range(32):
            ps1 = psA.tile([M, 512], F32, tag="ps1")
            nc.tensor.matmul(
                ps1[:], w1te_sb[:], x_aug[:, j * 512 : (j + 1) * 512],
                start=True, stop=True,
            )
            nc.scalar.activation(
                _ap(R, (4 * j + 1) * 130 + 1, [[G, M], [130, 4], [1, W]]),
                ps1[:],
                mybir.ActivationFunctionType.Relu,
            )
        psA.close()

        # ---- X^T: 128 matmuls (8 per PSUM bank) -> XE0 ----
        psX = _Pool(tc, name="psX", bufs=2, space="PSUM")
        for wb in range(16):
            pst = psX.tile([128, 512], F32, tag="pstx")
            for w8 in range(8):
                w = wb * 8 + w8
                nc.tensor.matmul(
                    pst[:, w8 * C : (w8 + 1) * C],
                    _ap(x_aug, w, [[HW, C], [W, H]]),
                    identc_sb[:],
                    start=True, stop=True,
                )
            nc.scalar.copy(
                _ap(XE0, 2 + wb * 8, [[XF, 128], [1, 8], [WP, C]]),
                _ap(pst, 0, [[512, 128], [C, 8], [1, C]]),
            )
        psX.close()

        # ---- conv3x3 + exp -> F ----
        psB = _Pool(tc, name="psB", bufs=2, space="PSUM")
        for j in range(32):
            ps2 = psB.tile([SK, 512], F32, tag="ps2")
            for t in range(9):
                ty, tx = divmod(t, 3)
                nc.tensor.matmul(
                    ps2[:],
                    wete_sb[:, t * SK : (t + 1) * SK],
                    _ap(R, (4 * j + ty) * 130 + tx, [[G, M + 1], [130, 4], [1, W]]),
                    start=(t == 0), stop=(t == 8),
                )
            nc.scalar.activation(
                F[:, j * 512 : (j + 1) * 512], ps2[:],
                mybir.ActivationFunctionType.Exp,
            )
        psB.close()
        pr.close()

        # ---- F^T + Z in one matmul per w-column (4 per PSUM bank) ----
        SZ = SK + S2
        psF = _Pool(tc, name="psF", bufs=4, space="PSUM")
        for wb in range(32):
            pst = psF.tile([128, 4 * SZ], F32, tag="pstf")
            for w4 in range(4):
                w = wb * 4 + w4
                nc.tensor.matmul(
                    pst[:, w4 * SZ : (w4 + 1) * SZ],
                    _ap(F, w, [[HW, SK], [W, H]]),
                    idents_sb[:],
                    start=True, stop=True,
                )
            nc.scalar.copy(
                _ap(fr, wb * 4, [[SK * W, 128], [1, 4], [W, SK]]),
                _ap(pst, 0, [[4 * SZ, 128], [SZ, 4], [1, SK]]),
            )
            nc.scalar.copy(
                _ap(rz, wb * 4, [[S2 * W, 128], [1, 4], [W, S2]]),
                _ap(pst, SK, [[4 * SZ, 128], [SZ, 4], [1, S2]]),
            )
        psF.close()
        pf.close()
        pxa.close()

        # ---- softmax normalize: fr *= 1/Z (broadcast over k) ----
        nc.vector.reciprocal(rz[:], rz[:])
        fr_bc = _ap(fr, 0, [[SK * W, 128], [K2 * W, S2], [W, K2], [1, W]])
        nc.vector.tensor_mul(
            fr_bc,
            fr_bc,
            _ap(rz, 0, [[S2 * W, 128], [W, S2], [0, K2], [1, W]]),
        )

        # ---- dy/parity shifted copies of XE0 (contiguous full rows) ----
        xeP = _Pool(tc, name="xeP", bufs=2)
        xoP = _Pool(tc, name="xoP", bufs=2)
        accP = _Pool(tc, name="acc", bufs=1)
        accs = [accP.tile([128, C * W], F16, tag=f"a{s}") for s in range(S2)]
        tmpP = _Pool(tc, name="tmp", bufs=1)
        tmp = tmpP.tile([128, C * W], F16, tag="tmp")
        acc2P = _Pool(tc, name="acc2", bufs=2)
        bufs = {}

        def zero_halo(X, dy):
            nh = abs(dy)
            p0h = 0 if dy < 0 else 128 - dy
            nc.sync.dma_start(
                _ap(X, p0h * XF, [[XF, nh], [1, XF]]), zer16.ap()[0:nh, :]
            )

        def issue_copies(dy):
            p0, p1 = max(0, -dy), 128 - max(0, dy)
            np_ = p1 - p0
            if dy == 0:
                XE = XE0
            else:
                XE = xeP.tile([128, XF], F16, tag="xe")
                nc.sync.dma_start(
                    _ap(XE, p0 * XF, [[XF, np_], [1, XF]]),
                    _ap(XE0, (p0 + dy) * XF, [[XF, np_], [1, XF]]),
                )
                zero_halo(XE, dy)
            XO = xoP.tile([128, XF], F16, tag="xo")
            nc.sync.dma_start(
                _ap(XO, p0 * XF, [[XF, np_], [1, XF - 1]]),
                _ap(XE0, (p0 + dy) * XF + 1, [[XF, np_], [1, XF - 1]]),
            )
            if dy != 0:
                zero_halo(XO, dy)
            bufs[dy] = (XE, XO)

        # ---- patch sum on VectorE ----
        DY_ORDER = [0, -1, 1, -2, 2]
        DX_ORDER = [-2, 0, 2, -1, 1]
        issue_copies(DY_ORDER[0])

        for di, dy in enumerate(DY_ORDER):
            if di + 1 < len(DY_ORDER):
                issue_copies(DY_ORDER[di + 1])
            XE, XO = bufs.pop(dy)
            for s in range(S2):
                for dx in DX_ORDER:
                    k = (dy + 2) * 5 + (dx + 2)
                    if dx % 2 == 0:
                        src, off = XE, 2 + dx
                    else:
                        src, off = XO, 1 + dx
                    in0 = _ap(src, off, [[XF, 128], [WP, C], [1, W]])
                    in1 = _ap(
                        fr, (s * K2 + k) * W, [[SK * W, 128], [0, C], [1, W]]
                    )
                    if dy == 0 and dx == DX_ORDER[0]:
                        dst = _ap(accs[s], 0, [[C * W, 128], [W, C], [1, W]])
                        nc.vector.tensor_mul(dst, in0, in1)
                    else:
                        dstt = _ap(tmp, 0, [[C * W, 128], [W, C], [1, W]])
                        nc.vector.tensor_mul(dstt, in0, in1)
                        nc.vector.tensor_add(accs[s][:], accs[s][:], tmp[:])

        # ---- pixel shuffle + store: per (s, eighth) ----
        for s in range(S2):
            for q in range(8):
                acc2 = acc2P.tile([128, 1024], F32, tag="acc2")
                # iteration (c4', r1, w, r2); src channel c = 8q + 4c4' + 2r1 + r2
                nc.scalar.copy(
                    _ap(acc2, 0, [[1024, 128], [512, 2], [256, 2], [2, W], [1, 2]]),
                    _ap(
                        accs[s],
                        8 * q * W,
                        [[C * W, 128], [4 * W, 2], [2 * W, 2], [1, W], [W, 2]],
                    ),
                )
                dst = bass.AP(
                    tensor=out,
                    offset=(s * 16 + 2 * q) * 4 * HW,
                    ap=[[2 * 2 * W, 128], [4 * HW, 2], [1, 512]],
                )
                nc.sync.dma_start(
                    dst, _ap(acc2, 0, [[1024, 128], [512, 2], [1, 512]])
                )
        acc2P.close()
        tmpP.close()
        accP.close()
        xoP.close()
        xeP.close()
        pxa_close = None  # (x_aug already closed above)
        prz.close()
        pfr.close()
        pxe0.close()
        cp.close()
    nc.compile()
    return nc


def host_inputs(x_img, w_compress, b_compress, w_encoder, b_encoder):
    """Per-core input map for one image [C, H, W]."""
    w1te = np.concatenate(
        [w_compress[:, :, 0, 0].T, b_compress[None, :]], axis=0
    ).astype(np.float16)
    wete = np.zeros((M + 1, 9, SK), np.float16)
    for ty in range(3):
        for tx in range(3):
            wete[:M, ty * 3 + tx, :] = w_encoder[:, :, ty, tx].T
    wete[M, 4, :] = b_encoder
    identc = np.eye(C, dtype=np.float16)
    idents = np.zeros((SK, SK + S2), np.float16)
    idents[:, :SK] = np.eye(SK)
    for s in range(S2):
        idents[s * K2 : (s + 1) * K2, SK + s] = 1.0
    return {
        "xin": np.ascontiguousarray(x_img.reshape(C, HW)).astype(np.float32),
        "w1te": w1te,
        "wete": wete.reshape(M + 1, 9 * SK),
        "identc": identc,
        "idents": idents,
        "ones16": np.ones((1, G), np.float16),
        "zer16": np.zeros((2, XF), np.float16),
    }


_CACHE = {}


def kernel(x, w_compress, b_compress, w_encoder, b_encoder):
    x = np.asarray(x, np.float32)
    if "nc" not in _CACHE:
        _CACHE["nc"] = build_program()
    nc = _CACHE["nc"]
    in_maps = [
        host_inputs(
            x[i],
            np.asarray(w_compress, np.float32),
            np.asarray(b_compress, np.float32),
            np.asarray(w_encoder, np.float32),
            np.asarray(b_encoder, np.float32),
        )
        for i in range(N_CORES)
    ]
    from concourse.bass_utils import run_bass_kernel_spmd

    res = run_bass_kernel_spmd(nc, in_maps, core_ids=list(range(N_CORES)))
    return np.stack(
        [res.results[i]["out"].reshape(C, 2 * H, 2 * W) for i in range(N_CORES)],
        axis=0,
    )


# revision 10
# speedup vs baseline: 3.3725x; 1.0109x over previous
"""CARAFE kernel for Trainium2 (8 NeuronCores, batch-parallel), v3.

Reference computation per image (one per core):
  R = relu(conv1x1(x, w_compress, b_compress))          [48, 128, 128]
  E = conv3x3(R, w_encoder, b_encoder, pad=1)           [100, 128, 128]
  Y = softmax over k of E.reshape(4, 25, H, W)          (s, k, h, w)
  out[s,c,h,w] = sum_k Y[s,k,h,w] * xpad[c, h+dy, w+dx] (k=(dy,dx), 5x5, pad 2)
  pixel-shuffle: out_ref[s*16 + c//4, 2h + (c//2)%2, 2w + c%2] = out[s,c,h,w]

Mapping (all 16-bit datapaths; measured E range is [-3.2, 3.3] so fp16
holds exp(E) and every intermediate comfortably):
  - conv1x1: fp16 matmuls, bias+relu fused in the PSUM->SBUF activation
    (bias as a per-partition vector).
  - conv3x3 over a zero-padded 130x130 R grid, 6 matmuls per 512-pixel
    chunk instead of 9: the R grid holds a second copy of R shifted by one
    element (rows 48-95, built by one SBUF->SBUF DMA), so taps (ty,0) and
    (ty,1) share a matmul with a 97-row contraction.  Bias via a ones row.
    exp fused into the PSUM->SBUF copy on ScalarE.
  - F^T transpose and the softmax denominator in ONE matmul per w-column:
    the moving operand is [I_100 | S] where S sums each s-group of 25, so
    PSUM gets F^T columns and Z^T columns together.  PSUM->SBUF epilogue
    copies run on VectorE (idle during the prefix); ScalarE keeps
    relu/exp/X^T epilogues.
  - X^T runs on the PE after conv3x3 (off the critical path to the
    normalize).  Its PSUM epilogue writes BOTH parity copies (XE0 and
    XO_0) so the dy=0 taps never wait on a DMA.
  - patch sum on VectorE in fp16 (2x perf mode): pixel-major layout
    [128 h-partitions, (c, w)].  dx taps are free-dim offsets; odd dx
    reads a one-element-shifted copy so every operand stays 4B-aligned.
    dy taps read partition-shifted copies of XE0 built by contiguous
    full-row SBUF->SBUF DMA into a 3-slot ring (+ the XO_0 tile), all
    prefetched one dy-phase ahead; edge partitions are zeroed (halo DMA).
  - pixel shuffle via a strided ScalarE copy (fp16->fp32) into
    (c4, r1, w, r2) order, then DMA with 2KB-contiguous runs.
"""

import sys

import numpy as np

sys.path.insert(0, "/opt/trn_rl_repo")

import concourse.bass as bass
import concourse.mybir as mybir
import concourse.tile as tile
from concourse import bacc

F32 = mybir.dt.float32
F16 = mybir.dt.float16

H = 128
W = 128
C = 64
M = 48  # compressed channels
S2 = 4  # scale_factor**2
K2 = 25  # k_up**2
SK = 100
HW = H * W
WP = 132  # padded row width in pixel-major x buffers
XF = C * WP  # 8448 free elems per partition
G = 130 * 130  # padded R grid
N_CORES = 8

# conv3x3 tap blocks: (moving ty, moving tx); rows 0-47 of the stationary
# hold tap (ty,tx), rows 48-95 hold tap (ty,tx+1) via the shifted R copy,
# row 96 rides the ones row (bias on block 1 only).
BLOCKS = [(0, 0, True), (1, 0, True), (2, 0, True), (0, 2, False), (1, 2, False), (2, 2, False)]


def _ap(t, extra_off, dims):
    """Raw AP on a tile handle `t` with free-offset `extra_off` (elements)
    and explicit [step, count] dims (dims[0] is the partition dim)."""
    base = t[:]
    return bass.AP(tensor=base.tensor, offset=base.offset + extra_off, ap=dims)


class _Pool:
    """Manually scoped tile pool."""

    def __init__(self, tc, **kw):
        self._cm = tc.tile_pool(**kw)
        self.pool = self._cm.__enter__()
        self._n = 0

    def tile(self, *a, tag=None, **kw):
        self._n += 1
        t = tag or f"t{self._n}"
        return self.pool.tile(*a, tag=t, name=t, **kw)

    def close(self):
        self._cm.__exit__(None, None, None)


def build_program():
    nc = bacc.Bacc("TRN2", target_bir_lowering=False, debug=False)

    xin = nc.dram_tensor("xin", [C, HW], F32, kind="ExternalInput")
    w1te = nc.dram_tensor("w1te", [C, M], F16, kind="ExternalInput")
    b1 = nc.dram_tensor("b1", [M, 1], F32, kind="ExternalInput")
    wete = nc.dram_tensor("wete", [M * 2 + 1, 6 * SK], F16, kind="ExternalInput")
    identc = nc.dram_tensor("identc", [C, C], F16, kind="ExternalInput")
    idents = nc.dram_tensor("idents", [SK, SK + S2], F16, kind="ExternalInput")
    ones16 = nc.dram_tensor("ones16", [1, G], F16, kind="ExternalInput")
    zer16 = nc.dram_tensor("zer16", [2, XF], F16, kind="ExternalInput")
    out = nc.dram_tensor("out", [C, 4 * HW], F32, kind="ExternalOutput")

    with tile.TileContext(nc) as tc:
        cp = _Pool(tc, name="consts", bufs=1)
        w1te_sb = cp.tile([C, M], F16, tag="w1te")
        nc.sync.dma_start(w1te_sb[:], w1te.ap())
        b1_sb = cp.tile([M, 1], F32, tag="b1")
        nc.sync.dma_start(b1_sb[:], b1.ap())
        wete_sb = cp.tile([M * 2 + 1, 6 * SK], F16, tag="wete")
        nc.sync.dma_start(wete_sb[:], wete.ap())
        identc_sb = cp.tile([C, C], F16, tag="identc")
        nc.sync.dma_start(identc_sb[:], identc.ap())
        idents_sb = cp.tile([SK, SK + S2], F16, tag="idents")
        nc.sync.dma_start(idents_sb[:], idents.ap())

        # persistent through the patch-sum phase
        pxe0 = _Pool(tc, name="pxe0", bufs=1)
        XE0 = pxe0.tile([128, XF], F16, tag="xe0")
        nc.gpsimd.memset(XE0[:], 0.0)
        pfr = _Pool(tc, name="pfr", bufs=1)
        fr = pfr.tile([128, SK * W], F16, tag="fr")
        prz = _Pool(tc, name="prz", bufs=1)
        rz = prz.tile([128, S2 * W], F32, tag="rz")
        pxo0 = _Pool(tc, name="pxo0", bufs=1)
        XO0 = pxo0.tile([128, XF], F16, tag="xo0")
        nc.gpsimd.memset(XO0[:], 0.0)

        pf = _Pool(tc, name="pf", bufs=1)
        F = pf.tile([SK, HW], F16, tag="F")

        # ---- load x (cast fp32->fp16 during DMA) ----
        pxa = _Pool(tc, name="pxa", bufs=1)
        x16 = pxa.tile([C, HW], F16, tag="x16")
        nc.gpsimd.dma_start(x16[:], xin.ap())

        # ---- R grid (with shifted duplicate rows 48-95, ones row 96) ----
        pr = _Pool(tc, name="pr", bufs=1)
        R = pr.tile([2 * M + 1, G], F16, tag="R")
        nc.gpsimd.memset(R[:], 0.0)
        nc.sync.dma_start(_ap(R, 2 * M * G, [[G, 1], [1, G]]), ones16.ap())

        # ---- conv1x1 + bias + relu into R interior ----
        psA = _Pool(tc, name="psA", bufs=3, space="PSUM")
        for j in range(32):
            ps1 = psA.tile([M, 512], F32, tag="ps1")
            nc.tensor.matmul(
                ps1[:], w1te_sb[:], x16[:, j * 512 : (j + 1) * 512],
                start=True, stop=True,
            )
            nc.scalar.activation(
                _ap(R, (4 * j + 1) * 130 + 1, [[G, M], [130, 4], [1, W]]),
                ps1[:],
                mybir.ActivationFunctionType.Relu,
                bias=b1_sb[:],
            )
        psA.close()

        # ---- duplicate R shifted by one element into rows 48-95 ----
        nc.sync.dma_start(
            _ap(R, M * G, [[G, M], [1, G - 1]]),
            _ap(R, 1, [[G, M], [1, G - 1]]),
        )

        # ---- conv3x3 (6 paired blocks) + exp -> F ----
        psB = _Pool(tc, name="psB", bufs=3, space="PSUM")
        for j in range(32):
            ps2 = psB.tile([SK, 512], F32, tag="ps2")
            for b, (ty, tx, _pair) in enumerate(BLOCKS):
                nc.tensor.matmul(
                    ps2[:],
                    wete_sb[:, b * SK : (b + 1) * SK],
                    _ap(R, (4 * j + ty) * 130 + tx, [[G, 2 * M + 1], [130, 4], [1, W]]),
                    start=(b == 0), stop=(b == len(BLOCKS) - 1),
                )
            nc.scalar.activation(
                F[:, j * 512 : (j + 1) * 512], ps2[:],
                mybir.ActivationFunctionType.Exp,
            )
        psB.close()
        pr.close()

        # ---- X^T: 128 matmuls (8 per PSUM bank) -> XE0 + XO0 ----
        psX = _Pool(tc, name="psX", bufs=3, space="PSUM")
        for wb in range(16):
            pst = psX.tile([128, 512], F32, tag="pstx")
            for w8 in range(8):
                w = wb * 8 + w8
                nc.tensor.matmul(
                    pst[:, w8 * C : (w8 + 1) * C],
                    _ap(x16, w, [[HW, C], [W, H]]),
                    identc_sb[:],
                    start=True, stop=True,
                )
            nc.scalar.copy(
                _ap(XE0, 2 + wb * 8, [[XF, 128], [1, 8], [WP, C]]),
                _ap(pst, 0, [[512, 128], [C, 8], [1, C]]),
            )
            nc.scalar.copy(
                _ap(XO0, 1 + wb * 8, [[XF, 128], [1, 8], [WP, C]]),
                _ap(pst, 0, [[512, 128], [C, 8], [1, C]]),
            )
        psX.close()
        pxa.close()

        # ---- F^T + Z in one matmul per w-column (4 per PSUM bank) ----
        SZ = SK + S2
        psF = _Pool(tc, name="psF", bufs=4, space="PSUM")
        for wb in range(32):
            pst = psF.tile([128, 4 * SZ], F32, tag="pstf")
            for w4 in range(4):
                w = wb * 4 + w4
                nc.tensor.matmul(
                    pst[:, w4 * SZ : (w4 + 1) * SZ],
                    _ap(F, w, [[HW, SK], [W, H]]),
                    idents_sb[:],
                    start=True, stop=True,
                )
            nc.vector.tensor_copy(
                _ap(fr, wb * 4, [[SK * W, 128], [1, 4], [W, SK]]),
                _ap(pst, 0, [[4 * SZ, 128], [SZ, 4], [1, SK]]),
            )
            nc.vector.tensor_copy(
                _ap(rz, wb * 4, [[S2 * W, 128], [1, 4], [W, S2]]),
                _ap(pst, SK, [[4 * SZ, 128], [SZ, 4], [1, S2]]),
            )
        psF.close()
        pf.close()

        # ---- softmax normalize: fr *= 1/Z (broadcast over k) ----
        nc.vector.reciprocal(rz[:], rz[:])
        fr_bc = _ap(fr, 0, [[SK * W, 128], [K2 * W, S2], [W, K2], [1, W]])
        nc.vector.tensor_mul(
            fr_bc,
            fr_bc,
            _ap(rz, 0, [[S2 * W, 128], [W, S2], [0, K2], [1, W]]),
        )

        # ---- patch-sum pools: 3-slot ring + XO0 for shifted x copies ----
        slotP = _Pool(tc, name="slots", bufs=1)
        slots = [slotP.tile([128, XF], F16, tag=f"sl{i}") for i in range(3)]
        accP = _Pool(tc, name="acc", bufs=1)
        accs = [accP.tile([128, C * W], F16, tag=f"a{s}") for s in range(S2)]
        tmpP = _Pool(tc, name="tmp", bufs=1)
        tmp = tmpP.tile([128, C * W], F16, tag="tmp")
        acc2P = _Pool(tc, name="acc2", bufs=2)

        # slot assignment per dy phase (current pair + prefetch pair alternate)
        slot_for = {-1: (slots[0], slots[1]), 1: (slots[2], XO0),
                    -2: (slots[0], slots[1]), 2: (slots[2], XO0)}
        bufs = {0: (XE0, XO0)}

        def issue_copies(dy):
            p0, p1 = max(0, -dy), 128 - max(0, dy)
            np_ = p1 - p0
            XE, XO = slot_for[dy]
            nc.sync.dma_start(
                _ap(XE, p0 * XF, [[XF, np_], [1, XF]]),
                _ap(XE0, (p0 + dy) * XF, [[XF, np_], [1, XF]]),
            )
            nc.sync.dma_start(
                _ap(XO, p0 * XF, [[XF, np_], [1, XF - 1]]),
                _ap(XE0, (p0 + dy) * XF + 1, [[XF, np_], [1, XF - 1]]),
            )
            nh = abs(dy)
            p0h = 0 if dy < 0 else 128 - dy
            for X in (XE, XO):
                nc.sync.dma_start(
                    _ap(X, p0h * XF, [[XF, nh], [1, XF]]), zer16.ap()[0:nh, :]
                )
            bufs[dy] = (XE, XO)

        # ---- patch sum on VectorE ----
        DY_ORDER = [0, -1, 1, -2, 2]
        DX_ORDER = [-2, 0, 2, -1, 1]

        for di, dy in enumerate(DY_ORDER):
            if di + 1 < len(DY_ORDER):
                issue_copies(DY_ORDER[di + 1])
            XE, XO = bufs.pop(dy)
            for s in range(S2):
                for dx in DX_ORDER:
                    k = (dy + 2) * 5 + (dx + 2)
                    if dx % 2 == 0:
                        src, off = XE, 2 + dx
                    else:
                        src, off = XO, 1 + dx
                    in0 = _ap(src, off, [[XF, 128], [WP, C], [1, W]])
                    in1 = _ap(
                        fr, (s * K2 + k) * W, [[SK * W, 128], [0, C], [1, W]]
                    )
                    if dy == 0 and dx == DX_ORDER[0]:
                        dst = _ap(accs[s], 0, [[C * W, 128], [W, C], [1, W]])
                        nc.vector.tensor_mul(dst, in0, in1)
                    else:
                        dstt = _ap(tmp, 0, [[C * W, 128], [W, C], [1, W]])
                        nc.vector.tensor_mul(dstt, in0, in1)
                        nc.vector.tensor_add(accs[s][:], accs[s][:], tmp[:])

        # ---- pixel shuffle + store: per (s, eighth) ----
        for s in range(S2):
            for q in range(8):
                acc2 = acc2P.tile([128, 1024], F32, tag="acc2")
                # iteration (c4', r1, w, r2); src channel c = 8q + 4c4' + 2r1 + r2
                nc.scalar.copy(
                    _ap(acc2, 0, [[1024, 128], [512, 2], [256, 2], [2, W], [1, 2]]),
                    _ap(
                        accs[s],
                        8 * q * W,
                        [[C * W, 128], [4 * W, 2], [2 * W, 2], [1, W], [W, 2]],
                    ),
                )
                dst = bass.AP(
                    tensor=out,
                    offset=(s * 16 + 2 * q) * 4 * HW,
                    ap=[[2 * 2 * W, 128], [4 * HW, 2], [1, 512]],
                )
                nc.sync.dma_start(
                    dst, _ap(acc2, 0, [[1024, 128], [512, 2], [1, 512]])
                )
        acc2P.close()
        tmpP.close()
        accP.close()
        slotP.close()
        pxo0.close()
        prz.close()
        pfr.close()
        pxe0.close()
        cp.close()
    nc.compile()
    return nc


def host_inputs(x_img, w_compress, b_compress, w_encoder, b_encoder):
    """Per-core input map for one image [C, H, W]."""
    w1te = w_compress[:, :, 0, 0].T.astype(np.float16)
    wete = np.zeros((2 * M + 1, 6, SK), np.float16)
    for b, (ty, tx, pair) in enumerate(BLOCKS):
        wete[:M, b, :] = w_encoder[:, :, ty, tx].T
        if pair:
            wete[M : 2 * M, b, :] = w_encoder[:, :, ty, tx + 1].T
    wete[2 * M, 1, :] = b_encoder
    identc = np.eye(C, dtype=np.float16)
    idents = np.zeros((SK, SK + S2), np.float16)
    idents[:, :SK] = np.eye(SK)
    for s in range(S2):
        idents[s * K2 : (s + 1) * K2, SK + s] = 1.0
    return {
        "xin": np.ascontiguousarray(x_img.reshape(C, HW)).astype(np.float32),
        "w1te": w1te,
        "b1": b_compress.reshape(M, 1).astype(np.float32),
        "wete": wete.reshape(2 * M + 1, 6 * SK),
        "identc": identc,
        "idents": idents,
        "ones16": np.ones((1, G), np.float16),
        "zer16": np.zeros((2, XF), np.float16),
    }


_CACHE = {}


def kernel(x, w_compress, b_compress, w_encoder, b_encoder):
    x = np.asarray(x, np.float32)
    if "nc" not in _CACHE:
        _CACHE["nc"] = build_program()
    nc = _CACHE["nc"]
    in_maps = [
        host_inputs(
            x[i],
            np.asarray(w_compress, np.float32),
            np.asarray(b_compress, np.float32),
            np.asarray(w_encoder, np.float32),
            np.asarray(b_encoder, np.float32),
        )
        for i in range(N_CORES)
    ]
    from concourse.bass_utils import run_bass_kernel_spmd

    res = run_bass_kernel_spmd(nc, in_maps, core_ids=list(range(N_CORES)))
    return np.stack(
        [res.results[i]["out"].reshape(C, 2 * H, 2 * W) for i in range(N_CORES)],
        axis=0,
    )


# revision 13
# speedup vs baseline: 3.7292x; 1.1058x over previous
"""CARAFE kernel for Trainium2 (8 NeuronCores, batch-parallel), v3.

Reference computation per image (one per core):
  R = relu(conv1x1(x, w_compress, b_compress))          [48, 128, 128]
  E = conv3x3(R, w_encoder, b_encoder, pad=1)           [100, 128, 128]
  Y = softmax over k of E.reshape(4, 25, H, W)          (s, k, h, w)
  out[s,c,h,w] = sum_k Y[s,k,h,w] * xpad[c, h+dy, w+dx] (k=(dy,dx), 5x5, pad 2)
  pixel-shuffle: out_ref[s*16 + c//4, 2h + (c//2)%2, 2w + c%2] = out[s,c,h,w]

Mapping (all 16-bit datapaths; measured E range is [-3.2, 3.3] so fp16
holds exp(E) and every intermediate comfortably):
  - conv1x1: fp16 matmuls, bias+relu fused in the PSUM->SBUF activation
    (bias as a per-partition vector).
  - conv3x3 over a zero-padded 130x130 R grid, 6 matmuls per 512-pixel
    chunk instead of 9: the R grid holds a second copy of R shifted by one
    element (rows 48-95, built by one SBUF->SBUF DMA), so taps (ty,0) and
    (ty,1) share a matmul with a 97-row contraction.  Bias via a ones row.
    exp fused into the PSUM->SBUF copy on ScalarE.
  - F^T transpose and the softmax denominator in ONE matmul per w-column:
    the moving operand is [I_100 | S] where S sums each s-group of 25, so
    PSUM gets F^T columns and Z^T columns together.  PSUM->SBUF epilogue
    copies run on VectorE (idle during the prefix); ScalarE keeps
    relu/exp/X^T epilogues.
  - X^T runs on the PE after conv3x3 (off the critical path to the
    normalize).  Its PSUM epilogue writes BOTH parity copies (XE0 and
    XO_0) so the dy=0 taps never wait on a DMA.
  - patch sum on VectorE in fp16 (2x perf mode): pixel-major layout
    [128 h-partitions, (c, w)].  dx taps are free-dim offsets; odd dx
    reads a one-element-shifted copy so every operand stays 4B-aligned.
    dy taps read partition-shifted copies of XE0 built by contiguous
    full-row SBUF->SBUF DMA into a 3-slot ring (+ the XO_0 tile), all
    prefetched one dy-phase ahead; edge partitions are zeroed (halo DMA).
  - pixel shuffle via a strided ScalarE copy (fp16->fp32) into
    (c4, r1, w, r2) order, then DMA with 2KB-contiguous runs.
"""

import sys

import numpy as np

sys.path.insert(0, "/opt/trn_rl_repo")

import concourse.bass as bass
import concourse.mybir as mybir
import concourse.tile as tile
from concourse import bacc

F32 = mybir.dt.float32
F16 = mybir.dt.float16

H = 128
W = 128
C = 64
M = 48  # compressed channels
S2 = 4  # scale_factor**2
K2 = 25  # k_up**2
SK = 100
HW = H * W
WP = 132  # padded row width in pixel-major x buffers
XF = C * WP  # 8448 free elems per partition
G = 130 * 130  # padded R grid
N_CORES = 8

# conv3x3 tap blocks: (moving ty, moving tx); rows 0-47 of the stationary
# hold tap (ty,tx), rows 48-95 hold tap (ty,tx+1) via the shifted R copy,
# row 96 rides the ones row (bias on block 1 only).
BLOCKS = [(0, 0, True), (1, 0, True), (2, 0, True), (0, 2, False), (1, 2, False), (2, 2, False)]


def _ap(t, extra_off, dims):
    """Raw AP on a tile handle `t` with free-offset `extra_off` (elements)
    and explicit [step, count] dims (dims[0] is the partition dim)."""
    base = t[:]
    return bass.AP(tensor=base.tensor, offset=base.offset + extra_off, ap=dims)


class _Pool:
    """Manually scoped tile pool."""

    def __init__(self, tc, **kw):
        self._cm = tc.tile_pool(**kw)
        self.pool = self._cm.__enter__()
        self._n = 0

    def tile(self, *a, tag=None, **kw):
        self._n += 1
        t = tag or f"t{self._n}"
        return self.pool.tile(*a, tag=t, name=t, **kw)

    def close(self):
        self._cm.__exit__(None, None, None)


def build_program():
    nc = bacc.Bacc("TRN2", target_bir_lowering=False, debug=False)

    xin = nc.dram_tensor("xin", [C, HW], F32, kind="ExternalInput")
    w1te = nc.dram_tensor("w1te", [C, M], F16, kind="ExternalInput")
    b1 = nc.dram_tensor("b1", [M, 1], F32, kind="ExternalInput")
    wete = nc.dram_tensor("wete", [M * 2 + 1, 6 * SK], F16, kind="ExternalInput")
    identc = nc.dram_tensor("identc", [C, C], F16, kind="ExternalInput")
    idents = nc.dram_tensor("idents", [SK, SK + S2], F16, kind="ExternalInput")
    ones16 = nc.dram_tensor("ones16", [1, G], F16, kind="ExternalInput")
    zer16 = nc.dram_tensor("zer16", [2, XF], F16, kind="ExternalInput")
    out = nc.dram_tensor("out", [C, 4 * HW], F32, kind="ExternalOutput")

    with tile.TileContext(nc) as tc:
        cp = _Pool(tc, name="consts", bufs=1)
        w1te_sb = cp.tile([C, M], F16, tag="w1te")
        nc.sync.dma_start(w1te_sb[:], w1te.ap())
        b1_sb = cp.tile([M, 1], F32, tag="b1")
        nc.sync.dma_start(b1_sb[:], b1.ap())
        wete_sb = cp.tile([M * 2 + 1, 6 * SK], F16, tag="wete")
        nc.sync.dma_start(wete_sb[:], wete.ap())
        identc_sb = cp.tile([C, C], F16, tag="identc")
        nc.sync.dma_start(identc_sb[:], identc.ap())
        idents_sb = cp.tile([SK, SK + S2], F16, tag="idents")
        nc.sync.dma_start(idents_sb[:], idents.ap())

        # persistent through the patch-sum phase
        pxe0 = _Pool(tc, name="pxe0", bufs=1)
        XE0 = pxe0.tile([128, XF], F16, tag="xe0")
        nc.gpsimd.memset(XE0[:], 0.0)
        pfr = _Pool(tc, name="pfr", bufs=1)
        fr = pfr.tile([128, SK * W], F16, tag="fr")
        prz = _Pool(tc, name="prz", bufs=1)
        rz = prz.tile([128, S2 * W], F32, tag="rz")
        pxo0 = _Pool(tc, name="pxo0", bufs=1)
        XO0 = pxo0.tile([128, XF], F16, tag="xo0")
        nc.gpsimd.memset(XO0[:], 0.0)

        # ---- load x (cast fp32->fp16 during DMA) ----
        pxa = _Pool(tc, name="pxa", bufs=1)
        x16 = pxa.tile([C, HW], F16, tag="x16")
        nc.gpsimd.dma_start(x16[:], xin.ap())

        pf = _Pool(tc, name="pf", bufs=1)
        F = pf.tile([SK, HW], F16, tag="F")

        # ---- R grid (with shifted duplicate rows 48-95, ones row 96) ----
        pr = _Pool(tc, name="pr", bufs=1)
        R = pr.tile([2 * M + 1, G], F16, tag="R")
        nc.gpsimd.memset(R[:], 0.0)
        nc.sync.dma_start(_ap(R, 2 * M * G, [[G, 1], [1, G]]), ones16.ap())

        # ---- conv1x1 + bias + relu into R interior ----
        psA = _Pool(tc, name="psA", bufs=4, space="PSUM")
        for j in range(32):
            ps1 = psA.tile([M, 512], F32, tag="ps1")
            nc.tensor.matmul(
                ps1[:], w1te_sb[:], x16[:, j * 512 : (j + 1) * 512],
                start=True, stop=True,
            )
            nc.vector.tensor_scalar(
                _ap(R, (4 * j + 1) * 130 + 1, [[G, M], [130, 4], [1, W]]),
                ps1[:],
                b1_sb[:],
                0.0,
                mybir.AluOpType.add,
                mybir.AluOpType.max,
            )
        psA.close()

        # ---- duplicate R shifted by one element into rows 48-95 ----
        nc.sync.dma_start(
            _ap(R, M * G, [[G, M], [1, G - 1]]),
            _ap(R, 1, [[G, M], [1, G - 1]]),
        )

        # ---- conv3x3 (6 paired blocks) + exp -> F ----
        psB = _Pool(tc, name="psB", bufs=4, space="PSUM")
        for j in range(32):
            ps2 = psB.tile([SK, 512], F32, tag="ps2")
            for b, (ty, tx, _pair) in enumerate(BLOCKS):
                nc.tensor.matmul(
                    ps2[:],
                    wete_sb[:, b * SK : (b + 1) * SK],
                    _ap(R, (4 * j + ty) * 130 + tx, [[G, 2 * M + 1], [130, 4], [1, W]]),
                    start=(b == 0), stop=(b == len(BLOCKS) - 1),
                )
            nc.scalar.activation(
                F[:, j * 512 : (j + 1) * 512], ps2[:],
                mybir.ActivationFunctionType.Exp,
            )
        psB.close()
        pr.close()

        # ---- F^T + Z in one matmul per w-column (4 per PSUM bank) ----
        SZ = SK + S2
        psF = _Pool(tc, name="psF", bufs=4, space="PSUM")
        for wb in range(32):
            pst = psF.tile([128, 4 * SZ], F32, tag="pstf")
            for w4 in range(4):
                w = wb * 4 + w4
                nc.tensor.matmul(
                    pst[:, w4 * SZ : (w4 + 1) * SZ],
                    _ap(F, w, [[HW, SK], [W, H]]),
                    idents_sb[:],
                    start=True, stop=True,
                )
            nc.vector.tensor_copy(
                _ap(fr, wb * 4, [[SK * W, 128], [W, SK], [1, 4]]),
                _ap(pst, 0, [[4 * SZ, 128], [1, SK], [SZ, 4]]),
            )
            nc.vector.tensor_copy(
                _ap(rz, wb * 4, [[S2 * W, 128], [W, S2], [1, 4]]),
                _ap(pst, SK, [[4 * SZ, 128], [1, S2], [SZ, 4]]),
            )
        psF.close()
        pf.close()

        # ---- X^T: 128 matmuls (8 per PSUM bank) -> XE0 + XO0 ----
        psX = _Pool(tc, name="psX", bufs=4, space="PSUM")
        for wb in range(16):
            pst = psX.tile([128, 512], F32, tag="pstx")
            for w8 in range(8):
                w = wb * 8 + w8
                nc.tensor.matmul(
                    pst[:, w8 * C : (w8 + 1) * C],
                    _ap(x16, w, [[HW, C], [W, H]]),
                    identc_sb[:],
                    start=True, stop=True,
                )
            nc.scalar.copy(
                _ap(XE0, 2 + wb * 8, [[XF, 128], [WP, C], [1, 8]]),
                _ap(pst, 0, [[512, 128], [1, C], [C, 8]]),
            )
            nc.scalar.copy(
                _ap(XO0, 1 + wb * 8, [[XF, 128], [WP, C], [1, 8]]),
                _ap(pst, 0, [[512, 128], [1, C], [C, 8]]),
            )
        psX.close()
        pxa.close()

        # ---- softmax normalize: fr *= 1/Z (broadcast over k) ----
        nc.vector.reciprocal(rz[:], rz[:])
        fr_bc = _ap(fr, 0, [[SK * W, 128], [K2 * W, S2], [W, K2], [1, W]])
        nc.vector.tensor_mul(
            fr_bc,
            fr_bc,
            _ap(rz, 0, [[S2 * W, 128], [W, S2], [0, K2], [1, W]]),
        )

        # ---- patch-sum pools: 3-slot ring + XO0 for shifted x copies ----
        slotP = _Pool(tc, name="slots", bufs=1)
        slots = [slotP.tile([128, XF], F16, tag=f"sl{i}") for i in range(3)]
        accP = _Pool(tc, name="acc", bufs=1)
        accs = [accP.tile([128, C * W], F16, tag=f"a{s}") for s in range(S2)]
        tmpP = _Pool(tc, name="tmp", bufs=1)
        tmp = tmpP.tile([128, C * W], F16, tag="tmp")
        acc2P = _Pool(tc, name="acc2", bufs=2)

        # slot assignment per dy phase (current pair + prefetch pair alternate)
        slot_for = {-1: (slots[0], slots[1]), 1: (slots[2], XO0),
                    -2: (slots[0], slots[1]), 2: (slots[2], XO0)}
        bufs = {0: (XE0, XO0)}

        def issue_copies(dy):
            p0, p1 = max(0, -dy), 128 - max(0, dy)
            np_ = p1 - p0
            XE, XO = slot_for[dy]
            nc.sync.dma_start(
                _ap(XE, p0 * XF, [[XF, np_], [1, XF]]),
                _ap(XE0, (p0 + dy) * XF, [[XF, np_], [1, XF]]),
            )
            nc.sync.dma_start(
                _ap(XO, p0 * XF, [[XF, np_], [1, XF - 1]]),
                _ap(XE0, (p0 + dy) * XF + 1, [[XF, np_], [1, XF - 1]]),
            )
            nh = abs(dy)
            p0h = 0 if dy < 0 else 128 - dy
            for X in (XE, XO):
                nc.sync.dma_start(
                    _ap(X, p0h * XF, [[XF, nh], [1, XF]]), zer16.ap()[0:nh, :]
                )
            bufs[dy] = (XE, XO)

        # ---- pixel shuffle + store: per (s, eighth) ----
        def store_s(s):
            for q in range(8):
                acc2 = acc2P.tile([128, 1024], F32, tag="acc2")
                # iteration (c4', r1, w, r2); src channel c = 8q + 4c4' + 2r1 + r2
                nc.scalar.copy(
                    _ap(acc2, 0, [[1024, 128], [512, 2], [256, 2], [2, W], [1, 2]]),
                    _ap(
                        accs[s],
                        8 * q * W,
                        [[C * W, 128], [4 * W, 2], [2 * W, 2], [1, W], [W, 2]],
                    ),
                )
                dst = bass.AP(
                    tensor=out,
                    offset=(s * 16 + 2 * q) * 4 * HW,
                    ap=[[2 * 2 * W, 128], [4 * HW, 2], [1, 512]],
                )
                nc.sync.dma_start(
                    dst, _ap(acc2, 0, [[1024, 128], [512, 2], [1, 512]])
                )

        # ---- patch sum on VectorE ----
        DY_ORDER = [0, -1, 1, -2, 2]
        DX_ORDER = [-2, 0, 2, -1, 1]

        for di, dy in enumerate(DY_ORDER):
            if di + 1 < len(DY_ORDER):
                issue_copies(DY_ORDER[di + 1])
            XE, XO = bufs.pop(dy)
            for s in range(S2):
                for dx in DX_ORDER:
                    k = (dy + 2) * 5 + (dx + 2)
                    if dx % 2 == 0:
                        src, off = XE, 2 + dx
                    else:
                        src, off = XO, 1 + dx
                    in0 = _ap(src, off, [[XF, 128], [WP, C], [1, W]])
                    in1 = _ap(
                        fr, (s * K2 + k) * W, [[SK * W, 128], [0, C], [1, W]]
                    )
                    if dy == 0 and dx == DX_ORDER[0]:
                        dst = _ap(accs[s], 0, [[C * W, 128], [W, C], [1, W]])
                        nc.vector.tensor_mul(dst, in0, in1)
                    else:
                        dstt = _ap(tmp, 0, [[C * W, 128], [W, C], [1, W]])
                        nc.vector.tensor_mul(dstt, in0, in1)
                        nc.vector.tensor_add(accs[s][:], accs[s][:], tmp[:])
                if dy == DY_ORDER[-1]:
                    store_s(s)

        acc2P.close()
        tmpP.close()
        accP.close()
        slotP.close()
        pxo0.close()
        prz.close()
        pfr.close()
        pxe0.close()
        cp.close()
    nc.compile()
    return nc


def host_inputs(x_img, w_compress, b_compress, w_encoder, b_encoder):
    """Per-core input map for one image [C, H, W]."""
    w1te = w_compress[:, :, 0, 0].T.astype(np.float16)
    wete = np.zeros((2 * M + 1, 6, SK), np.float16)
    for b, (ty, tx, pair) in enumerate(BLOCKS):
        wete[:M, b, :] = w_encoder[:, :, ty, tx].T
        if pair:
            wete[M : 2 * M, b, :] = w_encoder[:, :, ty, tx + 1].T
    wete[2 * M, 1, :] = b_encoder
    identc = np.eye(C, dtype=np.float16)
    idents = np.zeros((SK, SK + S2), np.float16)
    idents[:, :SK] = np.eye(SK)
    for s in range(S2):
        idents[s * K2 : (s + 1) * K2, SK + s] = 1.0
    return {
        "xin": np.ascontiguousarray(x_img.reshape(C, HW)).astype(np.float32),
        "w1te": w1te,
        "b1": b_compress.reshape(M, 1).astype(np.float32),
        "wete": wete.reshape(2 * M + 1, 6 * SK),
        "identc": identc,
        "idents": idents,
        "ones16": np.ones((1, G), np.float16),
        "zer16": np.zeros((2, XF), np.float16),
    }


_CACHE = {}


def kernel(x, w_compress, b_compress, w_encoder, b_encoder):
    x = np.asarray(x, np.float32)
    if "nc" not in _CACHE:
        _CACHE["nc"] = build_program()
    nc = _CACHE["nc"]
    in_maps = [
        host_inputs(
            x[i],
            np.asarray(w_compress, np.float32),
            np.asarray(b_compress, np.float32),
            np.asarray(w_encoder, np.float32),
            np.asarray(b_encoder, np.float32),
        )
        for i in range(N_CORES)
    ]
    from concourse.bass_utils import run_bass_kernel_spmd

    res = run_bass_kernel_spmd(nc, in_maps, core_ids=list(range(N_CORES)))
    return np.stack(
        [res.results[i]["out"].reshape(C, 2 * H, 2 * W) for i in range(N_CORES)],
        axis=0,
    )


# revision 15
# speedup vs baseline: 3.8118x; 1.0221x over previous
"""CARAFE kernel for Trainium2 (8 NeuronCores, batch-parallel), v3.

Reference computation per image (one per core):
  R = relu(conv1x1(x, w_compress, b_compress))          [48, 128, 128]
  E = conv3x3(R, w_encoder, b_encoder, pad=1)           [100, 128, 128]
  Y = softmax over k of E.reshape(4, 25, H, W)          (s, k, h, w)
  out[s,c,h,w] = sum_k Y[s,k,h,w] * xpad[c, h+dy, w+dx] (k=(dy,dx), 5x5, pad 2)
  pixel-shuffle: out_ref[s*16 + c//4, 2h + (c//2)%2, 2w + c%2] = out[s,c,h,w]

Mapping (all 16-bit datapaths; measured E range is [-3.2, 3.3] so fp16
holds exp(E) and every intermediate comfortably):
  - conv1x1: fp16 matmuls, bias+relu fused in the PSUM->SBUF activation
    (bias as a per-partition vector).
  - conv3x3 over a zero-padded 130x130 R grid, 6 matmuls per 512-pixel
    chunk instead of 9: the R grid holds a second copy of R shifted by one
    element (rows 48-95, built by one SBUF->SBUF DMA), so taps (ty,0) and
    (ty,1) share a matmul with a 97-row contraction.  Bias via a ones row.
    exp fused into the PSUM->SBUF copy on ScalarE.
  - F^T transpose and the softmax denominator in ONE matmul per w-column:
    the moving operand is [I_100 | S] where S sums each s-group of 25, so
    PSUM gets F^T columns and Z^T columns together.  PSUM->SBUF epilogue
    copies run on VectorE (idle during the prefix); ScalarE keeps
    relu/exp/X^T epilogues.
  - X^T runs on the PE after conv3x3 (off the critical path to the
    normalize).  Its PSUM epilogue writes BOTH parity copies (XE0 and
    XO_0) so the dy=0 taps never wait on a DMA.
  - patch sum on VectorE in fp16 (2x perf mode): pixel-major layout
    [128 h-partitions, (c, w)].  dx taps are free-dim offsets; odd dx
    reads a one-element-shifted copy so every operand stays 4B-aligned.
    dy taps read partition-shifted copies of XE0 built by contiguous
    full-row SBUF->SBUF DMA into a 3-slot ring (+ the XO_0 tile), all
    prefetched one dy-phase ahead; edge partitions are zeroed (halo DMA).
  - pixel shuffle via a strided ScalarE copy (fp16->fp32) into
    (c4, r1, w, r2) order, then DMA with 2KB-contiguous runs.
"""

import sys

import numpy as np

sys.path.insert(0, "/opt/trn_rl_repo")

import concourse.bass as bass
import concourse.mybir as mybir
import concourse.tile as tile
from concourse import bacc

F32 = mybir.dt.float32
F16 = mybir.dt.float16

H = 128
W = 128
C = 64
M = 48  # compressed channels
S2 = 4  # scale_factor**2
K2 = 25  # k_up**2
SK = 100
HW = H * W
WP = 132  # padded row width in pixel-major x buffers
XF = C * WP  # 8448 free elems per partition
G = 130 * 130  # padded R grid
N_CORES = 8

# conv3x3 tap blocks: (moving ty, moving tx); rows 0-47 of the stationary
# hold tap (ty,tx), rows 48-95 hold tap (ty,tx+1) via the shifted R copy,
# row 96 rides the ones row (bias on block 1 only).
BLOCKS = [(0, 0, True), (1, 0, True), (2, 0, True), (0, 2, False), (1, 2, False), (2, 2, False)]


def _ap(t, extra_off, dims):
    """Raw AP on a tile handle `t` with free-offset `extra_off` (elements)
    and explicit [step, count] dims (dims[0] is the partition dim)."""
    base = t[:]
    return bass.AP(tensor=base.tensor, offset=base.offset + extra_off, ap=dims)


class _Pool:
    """Manually scoped tile pool."""

    def __init__(self, tc, **kw):
        self._cm = tc.tile_pool(**kw)
        self.pool = self._cm.__enter__()
        self._n = 0

    def tile(self, *a, tag=None, **kw):
        self._n += 1
        t = tag or f"t{self._n}"
        return self.pool.tile(*a, tag=t, name=t, **kw)

    def close(self):
        self._cm.__exit__(None, None, None)


def build_program():
    nc = bacc.Bacc("TRN2", target_bir_lowering=False, debug=False)

    xin = nc.dram_tensor("xin", [C, HW], F32, kind="ExternalInput")
    w1te = nc.dram_tensor("w1te", [C, M], F16, kind="ExternalInput")
    b1 = nc.dram_tensor("b1", [M, 1], F32, kind="ExternalInput")
    wete = nc.dram_tensor("wete", [M * 2 + 1, 6 * SK], F16, kind="ExternalInput")
    identc = nc.dram_tensor("identc", [C, C], F16, kind="ExternalInput")
    idents = nc.dram_tensor("idents", [SK, SK + S2], F16, kind="ExternalInput")
    ones16 = nc.dram_tensor("ones16", [1, G], F16, kind="ExternalInput")
    zer16 = nc.dram_tensor("zer16", [2, XF], F16, kind="ExternalInput")
    out = nc.dram_tensor("out", [C, 4 * HW], F32, kind="ExternalOutput")

    with tile.TileContext(nc) as tc:
        cp = _Pool(tc, name="consts", bufs=1)
        w1te_sb = cp.tile([C, M], F16, tag="w1te")
        nc.sync.dma_start(w1te_sb[:], w1te.ap())
        b1_sb = cp.tile([M, 1], F32, tag="b1")
        nc.sync.dma_start(b1_sb[:], b1.ap())
        wete_sb = cp.tile([M * 2 + 1, 6 * SK], F16, tag="wete")
        nc.sync.dma_start(wete_sb[:], wete.ap())
        identc_sb = cp.tile([C, C], F16, tag="identc")
        nc.sync.dma_start(identc_sb[:], identc.ap())
        idents_sb = cp.tile([SK, SK + S2], F16, tag="idents")
        nc.sync.dma_start(idents_sb[:], idents.ap())

        # persistent through the patch-sum phase
        pxe0 = _Pool(tc, name="pxe0", bufs=1)
        XE0 = pxe0.tile([128, XF], F16, tag="xe0")
        nc.gpsimd.memset(XE0[:], 0.0)
        pfr = _Pool(tc, name="pfr", bufs=1)
        fr = pfr.tile([128, SK * W], F16, tag="fr")
        prz = _Pool(tc, name="prz", bufs=1)
        rz = prz.tile([128, S2 * W], F32, tag="rz")
        pxo0 = _Pool(tc, name="pxo0", bufs=1)
        XO0 = pxo0.tile([128, XF], F16, tag="xo0")
        nc.gpsimd.memset(XO0[:], 0.0)

        # ---- load x (cast fp32->fp16 during DMA) ----
        pxa = _Pool(tc, name="pxa", bufs=1)
        x16 = pxa.tile([C, HW], F16, tag="x16")
        nc.gpsimd.dma_start(x16[:], xin.ap())

        pf = _Pool(tc, name="pf", bufs=1)
        F = pf.tile([SK, HW], F16, tag="F")

        # ---- R grid (with shifted duplicate rows 48-95, ones row 96) ----
        pr = _Pool(tc, name="pr", bufs=1)
        R = pr.tile([2 * M + 1, G], F16, tag="R")
        nc.gpsimd.memset(R[:], 0.0)
        nc.sync.dma_start(_ap(R, 2 * M * G, [[G, 1], [1, G]]), ones16.ap())

        # ---- conv1x1 + bias + relu into R interior ----
        psA = _Pool(tc, name="psA", bufs=4, space="PSUM")
        for j in range(32):
            ps1 = psA.tile([M, 512], F32, tag="ps1")
            nc.tensor.matmul(
                ps1[:], w1te_sb[:], x16[:, j * 512 : (j + 1) * 512],
                start=True, stop=True,
            )
            nc.vector.tensor_scalar(
                _ap(R, (4 * j + 1) * 130 + 1, [[G, M], [130, 4], [1, W]]),
                ps1[:],
                b1_sb[:],
                0.0,
                mybir.AluOpType.add,
                mybir.AluOpType.max,
            )
        psA.close()

        # ---- duplicate R shifted by one element into rows 48-95 ----
        nc.sync.dma_start(
            _ap(R, M * G, [[G, M], [1, G - 1]]),
            _ap(R, 1, [[G, M], [1, G - 1]]),
        )

        # ---- conv3x3 (6 paired blocks) + exp -> F ----
        psB = _Pool(tc, name="psB", bufs=4, space="PSUM")
        for j in range(32):
            ps2 = psB.tile([SK, 512], F32, tag="ps2")
            for b, (ty, tx, _pair) in enumerate(BLOCKS):
                nc.tensor.matmul(
                    ps2[:],
                    wete_sb[:, b * SK : (b + 1) * SK],
                    _ap(R, (4 * j + ty) * 130 + tx, [[G, 2 * M + 1], [130, 4], [1, W]]),
                    start=(b == 0), stop=(b == len(BLOCKS) - 1),
                )
            nc.scalar.activation(
                F[:, j * 512 : (j + 1) * 512], ps2[:],
                mybir.ActivationFunctionType.Exp,
            )
        psB.close()
        pr.close()

        # ---- F^T + Z in one matmul per w-column (4 per PSUM bank) ----
        SZ = SK + S2
        psF = _Pool(tc, name="psF", bufs=4, space="PSUM")
        for wb in range(32):
            pst = psF.tile([128, 4 * SZ], F32, tag="pstf")
            for w4 in range(4):
                w = wb * 4 + w4
                nc.tensor.matmul(
                    pst[:, w4 * SZ : (w4 + 1) * SZ],
                    _ap(F, w, [[HW, SK], [W, H]]),
                    idents_sb[:],
                    start=True, stop=True,
                )
            nc.vector.tensor_copy(
                _ap(fr, wb * 4, [[SK * W, 128], [W, SK], [1, 4]]),
                _ap(pst, 0, [[4 * SZ, 128], [1, SK], [SZ, 4]]),
            )
            nc.vector.tensor_copy(
                _ap(rz, wb * 4, [[S2 * W, 128], [W, S2], [1, 4]]),
                _ap(pst, SK, [[4 * SZ, 128], [1, S2], [SZ, 4]]),
            )
        psF.close()
        pf.close()

        # ---- X^T: 128 matmuls (8 per PSUM bank) -> XE0 + XO0 ----
        psX = _Pool(tc, name="psX", bufs=4, space="PSUM")
        for wb in range(16):
            pst = psX.tile([128, 512], F32, tag="pstx")
            for w8 in range(8):
                w = wb * 8 + w8
                nc.tensor.matmul(
                    pst[:, w8 * C : (w8 + 1) * C],
                    _ap(x16, w, [[HW, C], [W, H]]),
                    identc_sb[:],
                    start=True, stop=True,
                )
            nc.scalar.copy(
                _ap(XE0, 2 + wb * 8, [[XF, 128], [WP, C], [1, 8]]),
                _ap(pst, 0, [[512, 128], [1, C], [C, 8]]),
            )
            nc.scalar.copy(
                _ap(XO0, 1 + wb * 8, [[XF, 128], [WP, C], [1, 8]]),
                _ap(pst, 0, [[512, 128], [1, C], [C, 8]]),
            )
        psX.close()
        pxa.close()

        # ---- softmax normalize: fr *= 1/Z (broadcast over k) ----
        nc.vector.reciprocal(rz[:], rz[:])
        fr_bc = _ap(fr, 0, [[SK * W, 128], [K2 * W, S2], [W, K2], [1, W]])
        nc.vector.tensor_mul(
            fr_bc,
            fr_bc,
            _ap(rz, 0, [[S2 * W, 128], [W, S2], [0, K2], [1, W]]),
        )

        # ---- patch-sum pools: 3-slot ring + XO0 for shifted x copies ----
        slotP = _Pool(tc, name="slots", bufs=1)
        slots = [slotP.tile([128, XF], F16, tag=f"sl{i}") for i in range(3)]
        accP = _Pool(tc, name="acc", bufs=1)
        accs = [accP.tile([128, C * W], F16, tag=f"a{s}") for s in range(S2)]
        tmpP = _Pool(tc, name="tmp", bufs=1)
        tmp = tmpP.tile([128, C * W], F16, tag="tmp")
        acc2P = _Pool(tc, name="acc2", bufs=2)

        # slot assignment per dy phase (current pair + prefetch pair alternate)
        slot_for = {-1: (slots[0], slots[1]), 1: (slots[2], XO0),
                    -2: (slots[0], slots[1]), 2: (slots[2], XO0)}
        bufs = {0: (XE0, XO0)}

        def issue_copies(dy):
            p0, p1 = max(0, -dy), 128 - max(0, dy)
            XE, XO = slot_for[dy]
            # split each body copy into 4 partition chunks on alternating
            # HWDGE queues: one dma_start lands on ONE SDMA engine (~27GB/s),
            # so chunking is what buys DMA parallelism.
            for X, eoff in ((XE, 0), (XO, 1)):
                bounds = [p0 + (p1 - p0) * i // 4 for i in range(5)]
                for c0, c1 in zip(bounds, bounds[1:]):
                    eng = nc.sync if (eoff + c0) % 2 == 0 else nc.scalar
                    eng.dma_start(
                        _ap(X, c0 * XF, [[XF, c1 - c0], [1, XF - eoff]]),
                        _ap(XE0, (c0 + dy) * XF + eoff, [[XF, c1 - c0], [1, XF - eoff]]),
                    )
            nh = abs(dy)
            p0h = 0 if dy < 0 else 128 - dy
            for X in (XE, XO):
                nc.sync.dma_start(
                    _ap(X, p0h * XF, [[XF, nh], [1, XF]]), zer16.ap()[0:nh, :]
                )
            bufs[dy] = (XE, XO)

        # ---- pixel shuffle + store: per (s, eighth) ----
        def store_s(s):
            for q in range(8):
                acc2 = acc2P.tile([128, 1024], F32, tag="acc2")
                # iteration (c4', r1, w, r2); src channel c = 8q + 4c4' + 2r1 + r2
                nc.scalar.copy(
                    _ap(acc2, 0, [[1024, 128], [512, 2], [256, 2], [2, W], [1, 2]]),
                    _ap(
                        accs[s],
                        8 * q * W,
                        [[C * W, 128], [4 * W, 2], [2 * W, 2], [1, W], [W, 2]],
                    ),
                )
                for c4 in range(2):
                    dst = bass.AP(
                        tensor=out,
                        offset=(s * 16 + 2 * q + c4) * 4 * HW,
                        ap=[[2 * 2 * W, 128], [1, 512]],
                    )
                    eng = nc.sync if (q + c4) % 2 == 0 else nc.scalar
                    eng.dma_start(
                        dst, _ap(acc2, c4 * 512, [[1024, 128], [1, 512]])
                    )

        # ---- patch sum on VectorE ----
        DY_ORDER = [0, -1, 1, -2, 2]
        DX_ORDER = [-2, 0, 2, -1, 1]

        for di, dy in enumerate(DY_ORDER):
            if di + 1 < len(DY_ORDER):
                issue_copies(DY_ORDER[di + 1])
            XE, XO = bufs.pop(dy)
            for s in range(S2):
                for dx in DX_ORDER:
                    k = (dy + 2) * 5 + (dx + 2)
                    if dx % 2 == 0:
                        src, off = XE, 2 + dx
                    else:
                        src, off = XO, 1 + dx
                    in0 = _ap(src, off, [[XF, 128], [WP, C], [1, W]])
                    in1 = _ap(
                        fr, (s * K2 + k) * W, [[SK * W, 128], [0, C], [1, W]]
                    )
                    if dy == 0 and dx == DX_ORDER[0]:
                        dst = _ap(accs[s], 0, [[C * W, 128], [W, C], [1, W]])
                        nc.vector.tensor_mul(dst, in0, in1)
                    else:
                        dstt = _ap(tmp, 0, [[C * W, 128], [W, C], [1, W]])
                        nc.vector.tensor_mul(dstt, in0, in1)
                        nc.vector.tensor_add(accs[s][:], accs[s][:], tmp[:])
                if dy == DY_ORDER[-1]:
                    store_s(s)

        acc2P.close()
        tmpP.close()
        accP.close()
        slotP.close()
        pxo0.close()
        prz.close()
        pfr.close()
        pxe0.close()
        cp.close()
    nc.compile()
    return nc


def host_inputs(x_img, w_compress, b_compress, w_encoder, b_encoder):
    """Per-core input map for one image [C, H, W]."""
    w1te = w_compress[:, :, 0, 0].T.astype(np.float16)
    wete = np.zeros((2 * M + 1, 6, SK), np.float16)
    for b, (ty, tx, pair) in enumerate(BLOCKS):
        wete[:M, b, :] = w_encoder[:, :, ty, tx].T
        if pair:
            wete[M : 2 * M, b, :] = w_encoder[:, :, ty, tx + 1].T
    wete[2 * M, 1, :] = b_encoder
    identc = np.eye(C, dtype=np.float16)
    idents = np.zeros((SK, SK + S2), np.float16)
    idents[:, :SK] = np.eye(SK)
    for s in range(S2):
        idents[s * K2 : (s + 1) * K2, SK + s] = 1.0
    return {
        "xin": np.ascontiguousarray(x_img.reshape(C, HW)).astype(np.float32),
        "w1te": w1te,
        "b1": b_compress.reshape(M, 1).astype(np.float32),
        "wete": wete.reshape(2 * M + 1, 6 * SK),
        "identc": identc,
        "idents": idents,
        "ones16": np.ones((1, G), np.float16),
        "zer16": np.zeros((2, XF), np.float16),
    }


_CACHE = {}


def kernel(x, w_compress, b_compress, w_encoder, b_encoder):
    x = np.asarray(x, np.float32)
    if "nc" not in _CACHE:
        _CACHE["nc"] = build_program()
    nc = _CACHE["nc"]
    in_maps = [
        host_inputs(
            x[i],
            np.asarray(w_compress, np.float32),
            np.asarray(b_compress, np.float32),
            np.asarray(w_encoder, np.float32),
            np.asarray(b_encoder, np.float32),
        )
        for i in range(N_CORES)
    ]
    from concourse.bass_utils import run_bass_kernel_spmd

    res = run_bass_kernel_spmd(nc, in_maps, core_ids=list(range(N_CORES)))
    return np.stack(
        [res.results[i]["out"].reshape(C, 2 * H, 2 * W) for i in range(N_CORES)],
        axis=0,
    )
